# revision 4
# baseline (speedup 1.0000x reference)
"""Trainium2 Bass kernel for nn_MultiHeadAttention_45672682226228.

The reference module computes multi-head attention but everything except the
V projection is dead code (DCE'd under jit): the returned value is

    out[b, s, 64*h + q] = x[b, s, 768 + 64*h + q]
                        + sum_d x[b, s, 256*h + d] * W_v[q, d]

i.e. a per-token block-diagonal matmul (4 heads x [256 -> 64]) plus a
residual add of the last head's input slice.  W_q / W_k are unused.

Kernel strategy (v2):
  * Data-parallel over batch B=16 -> 2 batches (8192 tokens) per core.
  * The residual is folded into the weights: W_big [1024, 256] =
    blockdiag(W_v.T per head) + rows 768:1024 get +I.  The whole module is
    then a single matmul  out = x @ W_big.
  * x is pre-transposed and cast to bf16 on the HOST, so the device streams
    xT [1024, 8192] tiles straight into accumulating PE matmuls
    outT[c,t] = sum_j W_big[j].T @ xT[j] - no on-chip transposes at all.
  * Per 512-token group: c-chunk 0 needs d-chunks {0,1,2,3,6}, c-chunk 1
    needs {4,5,6,7} (W_big is block-sparse) -> 9 matmuls of N=512.
  * Output is computed as outT [256, 8192] f32 in PSUM, cast to bf16 in
    SBUF (halves store traffic), DMA'd out, and un-transposed/upcast on the
    host.  bf16 end-to-end error ~5e-3, well inside the 2e-2 gate.

Per-core HBM traffic: 16 MiB in + 4 MiB out (vs 40 MiB for the f32
transpose-on-device kernel).
"""

import os
import numpy as np

P = 128
TPC = 8192          # tokens per core
NCORES = 8
TBLK = 2048         # tokens per DMA tile
NTB = TPC // TBLK   # 4
GRP = 512           # tokens per matmul group (PSUM bank = 512 f32)
NGRP = TBLK // GRP  # 4

# d-chunks feeding each 128-wide c-chunk of W_big (block sparsity)
CC_J = [[0, 1, 2, 3, 6], [4, 5, 6, 7]]
# per t-block DMA arrival order; matmuls are emitted tile-major in this same
# order so the PE consumes each tile the moment it lands and the last tile of
# the kernel gates only 4 matmuls (not a whole block's worth)
LOAD_ORDER = [6, 4, 5, 7, 0, 1, 2, 3]

_STATE = {}


def _bf16():
    import ml_dtypes

    return ml_dtypes.bfloat16


def _pack_wbig(W_v: np.ndarray) -> np.ndarray:
    """W_big [1024, 256] = blockdiag(W_v.T) + I on rows 768:1024.

    Packed as [p, j, c] (d-within-chunk, d-chunk, out-col) so the DMA into
    SBUF [128, 8, 256] is fully contiguous.
    """
    W_v = np.asarray(W_v, np.float32)
    Wb = np.zeros((1024, 256), np.float32)
    for h in range(4):
        Wb[256 * h:256 * (h + 1), 64 * h:64 * (h + 1)] = W_v.T
    Wb[np.arange(768, 1024), np.arange(256)] += 1.0
    pj = Wb.reshape(8, P, 256).transpose(1, 0, 2)
    return np.ascontiguousarray(pj).astype(_bf16())


def _build_nc(tpc=TPC):
    from contextlib import ExitStack

    import concourse.mybir as mybir
    import concourse.tile as tile
    from concourse import bacc
    from concourse.bass import ds, ts

    bf16 = mybir.dt.bfloat16
    f32 = mybir.dt.float32

    nc = bacc.Bacc("TRN2", target_bir_lowering=False, debug=False)
    xt_h = nc.dram_tensor("xt", [8, P, tpc], bf16, kind="ExternalInput")
    w_h = nc.dram_tensor("wbig", [P, 8, 256], bf16, kind="ExternalInput")
    o_h = nc.dram_tensor("out", [2, P, tpc], bf16, kind="ExternalOutput")

    ntb = tpc // TBLK

    with ExitStack() as ctx:
        tc = ctx.enter_context(tile.TileContext(nc))
        sb = ctx.enter_context(tc.tile_pool(name="sb", bufs=1))
        ps = ctx.enter_context(tc.tile_pool(name="ps", bufs=4, space="PSUM"))

        w_sb = sb.tile([P, 8, 256], bf16)
        nc.sync.dma_start(w_sb[:], w_h[:])

        xt_sb = sb.tile([P, 8, tpc], bf16)   # 128 KiB / partition
        out_sb = sb.tile([P, 2, tpc], bf16)  # 32 KiB / partition

        # Enqueue every input load up-front; the two HWDGE rings stream them
        # back-to-back at full rate while the PE chews through groups.
        # Alternate rings per tile (scalar first: sync already carries wbig)
        # so the two rings finish together.
        for tb in range(ntb):
            for i, j in enumerate(LOAD_ORDER):
                eng = nc.scalar if (tb * 8 + i) % 2 == 0 else nc.sync
                eng.dma_start(
                    xt_sb[:, j, ts(tb, TBLK)], xt_h[j, :, ts(tb, TBLK)]
                )

        # arrival position of each d-chunk within a block, per c-chunk:
        # the first-arriving chunk carries start=True, the last stop=True
        cc_order = [
            [j for j in LOAD_ORDER if j in CC_J[cc]] for cc in range(2)
        ]

        for tb in range(ntb):
            tsl = [ds(tb * TBLK + g * GRP, GRP) for g in range(NGRP)]
            pm = {
                (g, cc): ps.tile([P, GRP], f32, tag=f"pm{cc}", name=f"pm{cc}")
                for g in range(NGRP)
                for cc in range(2)
            }
            # tile-major: as tile (j, tb) lands, run its matmuls for all 4
            # groups immediately; each accumulates into its (g, cc) bank
            for j in LOAD_ORDER:
                for g in range(NGRP):
                    for cc in range(2):
                        if j not in CC_J[cc]:
                            continue
                        seq = cc_order[cc]
                        nc.tensor.matmul(
                            pm[(g, cc)][:],
                            w_sb[:, j, ts(cc, P)],
                            xt_sb[:, j, tsl[g]],
                            start=(j == seq[0]),
                            stop=(j == seq[-1]),
                        )
                # cast a (g, cc) bank out as soon as its last matmul ran
                for g in range(NGRP):
                    for cc in range(2):
                        if j == cc_order[cc][-1]:
                            if cc == 0:
                                nc.vector.tensor_copy(
                                    out_sb[:, 0, tsl[g]], pm[(g, 0)][:]
                                )
                            else:
                                nc.scalar.copy(
                                    out_sb[:, 1, tsl[g]], pm[(g, 1)][:]
                                )
            if tb < ntb - 1:
                # SWDGE so stores don't head-of-line block input HWDGE rings
                for cc in range(2):
                    nc.gpsimd.dma_start(
                        o_h[cc, :, ts(tb, TBLK)], out_sb[:, cc, ts(tb, TBLK)]
                    )
            else:
                # input rings are empty by now: ship the tail per-group on
                # the low-latency HWDGE rings to shrink the kernel tail
                for g in range(NGRP):
                    for cc in range(2):
                        eng = nc.sync if cc == 0 else nc.scalar
                        eng.dma_start(
                            o_h[cc, :, tsl[g]], out_sb[:, cc, tsl[g]]
                        )

    nc.compile()
    return nc


def _install_ntff_hook():
    """Provide antenv.axon_hooks (absent in this image) so trace=True works."""
    import sys
    import types

    if "antenv.axon_hooks" in sys.modules:
        return
    try:
        import trn_agent_boot.trn_boot as tb

        hook = tb._ntff_profile_via_ctypes("/opt/axon/libaxon_pjrt.so")
    except Exception:
        hook = None
    mod = types.ModuleType("antenv.axon_hooks")
    mod.get_axon_ntff_profile_hook = lambda: hook
    mod.set_axon_ntff_profile_hook = lambda h: None
    sys.modules["antenv.axon_hooks"] = mod
    try:
        import antenv

        antenv.axon_hooks = mod
    except ImportError:
        pass


def kernel(x, W_q=None, W_k=None, W_v=None, **_):
    from concourse.bass_utils import run_bass_kernel_spmd

    if "nc" not in _STATE:
        _STATE["nc"] = _build_nc()
    nc = _STATE["nc"]
    bf16 = _bf16()

    x = np.asarray(x, np.float32)
    b, s, e = x.shape
    xf = x.reshape(b * s, e).astype(bf16)  # one contiguous f32->bf16 pass
    wbig = _pack_wbig(W_v)

    in_maps = []
    for c in range(NCORES):
        xtc = np.ascontiguousarray(xf[c * TPC:(c + 1) * TPC].T)  # [1024, TPC]
        in_maps.append({"xt": xtc.reshape(8, P, TPC), "wbig": wbig})

    trace = os.environ.get("KERNEL_TRACE", "0") == "1"
    if trace:
        _install_ntff_hook()
    res = run_bass_kernel_spmd(nc, in_maps, core_ids=list(range(NCORES)), trace=trace)
    _STATE["last_results"] = res

    outs = []
    for r in res.results:
        oc = np.asarray(r["out"]).reshape(256, TPC)  # [c, t] bf16
        outs.append(oc.T.astype(np.float32))         # [t, c] f32
    out = np.concatenate(outs, axis=0)
    return out.reshape(b, s, 256)


# revision 5
# speedup vs baseline: 1.1375x; 1.1375x over previous
"""Trainium2 Bass kernel for nn_MultiHeadAttention_45672682226228.

The reference module computes multi-head attention but everything except the
V projection is dead code (DCE'd under jit): the returned value is

    out[b, s, 64*h + q] = x[b, s, 768 + 64*h + q]
                        + sum_d x[b, s, 256*h + d] * W_v[q, d]

i.e. a per-token block-diagonal matmul (4 heads x [256 -> 64]) plus a
residual add of the last head's input slice.  W_q / W_k are unused.

Kernel strategy (v4):
  * Data-parallel over batch B=16 -> 2 batches (8192 tokens) per core.
  * x is pre-transposed and cast to bf16 on the HOST, so the device streams
    xT [1024, 8192] = 8 chunks of [128, 8192] straight into accumulating PE
    matmuls - no on-chip transposes (bf16 error ~3e-3, gate is 2e-2).
  * All 4 heads share W_v, so the only weights are A = W_v.T[0:128] and
    B = W_v.T[128:256], both [128, 64].  M=64 means two matmuls are packed
    side-by-side in the PE array via column tiling (tile_position (0,0) /
    (0,64)), halving PE streaming time:
      outT[  0:128] (heads 0,1): (A@x0 || A@x2), (B@x1 || B@x3)
      outT[128:256] (heads 2,3): (A@x4 || A@x6), (B@x5 || B@x7),
                                 (D0@x7 || D1@x7)   <- residual cols 128:255
    The cc0 residual (xT rows 768:896 = chunk 6, partition-aligned with
    output cols 0:128) is added by the DVE during PSUM evacuation; the cc1
    residual rides two diagonal weight blocks so evacuation can use the
    Scalar engine (which cannot do two-tensor adds).
  * Matmuls are emitted tile-major: each input tile is consumed for all 4
    groups the moment it lands, so the last DMA gates only ~2 us of PE work.
  * outT is evacuated as bf16 (halves store traffic), un-transposed and
    upcast on the host.

Per-core HBM traffic: 16 MiB in + 4 MiB out; PE streams 5x512 columns per
512-token group.
"""

import os
import numpy as np

P = 128
TPC = 8192          # tokens per core
NCORES = 8
TBLK = 2048         # tokens per DMA tile
GRP = 512           # tokens per matmul group (PSUM bank = 512 f32)
NGRP = TBLK // GRP  # 4

# DMA arrival order of the 8 d-chunks within each t-block.  Matmul slots
# fire as soon as the later chunk of their pair lands:
#   j6 -> (A@4 || A@6) needs j4,j6 ; j7 -> (B@5 || B@7), (D@7 || D@7)
#   j2 -> (A@0 || A@2)             ; j3 -> (B@1 || B@3) + DVE residual add
LOAD_ORDER = [4, 6, 5, 7, 0, 2, 1, 3]

_STATE = {}


def _bf16():
    import ml_dtypes

    return ml_dtypes.bfloat16


def _pack_w(W_v: np.ndarray) -> np.ndarray:
    """Pack [128, 4, 64] bf16: A, B (shared by all heads), D0, D1 (diag)."""
    W_v = np.asarray(W_v, np.float32)
    w = np.zeros((P, 4, 64), np.float32)
    w[:, 0, :] = W_v.T[0:128]     # A
    w[:, 1, :] = W_v.T[128:256]   # B
    w[0:64, 2, :] = np.eye(64)    # D0: out cols 128:192 += xT rows 896:960
    w[64:128, 3, :] = np.eye(64)  # D1: out cols 192:256 += xT rows 960:1024
    return np.ascontiguousarray(w).astype(_bf16())


def _build_nc(tpc=TPC):
    from contextlib import ExitStack

    import concourse.mybir as mybir
    import concourse.tile as tile
    from concourse import bacc
    from concourse.bass import ds, ts

    bf16 = mybir.dt.bfloat16
    f32 = mybir.dt.float32

    nc = bacc.Bacc("TRN2", target_bir_lowering=False, debug=False)
    xt_h = nc.dram_tensor("xt", [8, P, tpc], bf16, kind="ExternalInput")
    w_h = nc.dram_tensor("w", [P, 4, 64], bf16, kind="ExternalInput")
    o_h = nc.dram_tensor("out", [2, P, tpc], bf16, kind="ExternalOutput")

    ntb = tpc // TBLK

    with ExitStack() as ctx:
        tc = ctx.enter_context(tile.TileContext(nc))
        sb = ctx.enter_context(tc.tile_pool(name="sb", bufs=1))
        ps = ctx.enter_context(tc.tile_pool(name="ps", bufs=4, space="PSUM"))

        w_sb = sb.tile([P, 4, 64], bf16)
        nc.sync.dma_start(w_sb[:], w_h[:])
        A, B, D0, D1 = (w_sb[:, k, :] for k in range(4))

        xt_sb = sb.tile([P, 8, tpc], bf16)   # 128 KiB / partition
        out_sb = sb.tile([P, 2, tpc], bf16)  # 32 KiB / partition

        # Enqueue every input load up-front; the two HWDGE rings stream them
        # back-to-back while the PE consumes tiles as they land.
        for tb in range(ntb):
            for i, j in enumerate(LOAD_ORDER):
                eng = nc.scalar if (tb * 8 + i) % 2 == 0 else nc.sync
                eng.dma_start(
                    xt_sb[:, j, ts(tb, TBLK)], xt_h[j, :, ts(tb, TBLK)]
                )

        def pair(pm, lhs0, j0, lhs1, j1, tsl, start, stop):
            nc.tensor.matmul(pm[0:64, :], lhs0, xt_sb[:, j0, tsl],
                             start=start, stop=stop, tile_position=(0, 0))
            nc.tensor.matmul(pm[64:128, :], lhs1, xt_sb[:, j1, tsl],
                             start=start, stop=stop, tile_position=(0, 64))

        for tb in range(ntb):
            tsl = [ds(tb * TBLK + g * GRP, GRP) for g in range(NGRP)]
            pm = {
                (g, cc): ps.tile([P, GRP], f32, tag=f"pm{cc}", name=f"pm{cc}")
                for g in range(NGRP)
                for cc in range(2)
            }
            for g in range(NGRP):  # after j4, j6 land
                pair(pm[(g, 1)], A, 4, A, 6, tsl[g], True, False)
            for g in range(NGRP):  # after j5, j7 land
                pair(pm[(g, 1)], B, 5, B, 7, tsl[g], False, False)
                pair(pm[(g, 1)], D0, 7, D1, 7, tsl[g], False, True)
                # heads 2,3 + residual done -> evacuate via ScalarE
                nc.scalar.copy(out_sb[:, 1, tsl[g]], pm[(g, 1)][:])
            for g in range(NGRP):  # after j0, j2 land
                pair(pm[(g, 0)], A, 0, A, 2, tsl[g], True, False)
            for g in range(NGRP):  # after j1, j3 land
                pair(pm[(g, 0)], B, 1, B, 3, tsl[g], False, True)
                # heads 0,1 + residual (xT chunk 6 is partition-aligned)
                nc.vector.tensor_add(
                    out_sb[:, 0, tsl[g]], pm[(g, 0)][:], xt_sb[:, 6, tsl[g]]
                )
            if tb < ntb - 1:
                # SWDGE so stores don't head-of-line block input HWDGE rings
                for cc in range(2):
                    nc.gpsimd.dma_start(
                        o_h[cc, :, ts(tb, TBLK)], out_sb[:, cc, ts(tb, TBLK)]
                    )
            else:
                # input rings are empty by now: ship the tail per-group on
                # the low-latency HWDGE rings to shrink the kernel tail
                for g in range(NGRP):
                    for cc in range(2):
                        eng = nc.sync if cc == 0 else nc.scalar
                        eng.dma_start(
                            o_h[cc, :, tsl[g]], out_sb[:, cc, tsl[g]]
                        )

    nc.compile()
    return nc


def _install_ntff_hook():
    """Provide antenv.axon_hooks (absent in this image) so trace=True works."""
    import sys
    import types

    if "antenv.axon_hooks" in sys.modules:
        return
    try:
        import trn_agent_boot.trn_boot as tb

        hook = tb._ntff_profile_via_ctypes("/opt/axon/libaxon_pjrt.so")
    except Exception:
        hook = None
    mod = types.ModuleType("antenv.axon_hooks")
    mod.get_axon_ntff_profile_hook = lambda: hook
    mod.set_axon_ntff_profile_hook = lambda h: None
    sys.modules["antenv.axon_hooks"] = mod
    try:
        import antenv

        antenv.axon_hooks = mod
    except ImportError:
        pass


def kernel(x, W_q=None, W_k=None, W_v=None, **_):
    from concourse.bass_utils import run_bass_kernel_spmd

    if "nc" not in _STATE:
        _STATE["nc"] = _build_nc()
    nc = _STATE["nc"]
    bf16 = _bf16()

    x = np.asarray(x, np.float32)
    b, s, e = x.shape
    xf = x.reshape(b * s, e).astype(bf16)  # one contiguous f32->bf16 pass
    w = _pack_w(W_v)

    in_maps = []
    for c in range(NCORES):
        xtc = np.ascontiguousarray(xf[c * TPC:(c + 1) * TPC].T)  # [1024, TPC]
        in_maps.append({"xt": xtc.reshape(8, P, TPC), "w": w})

    trace = os.environ.get("KERNEL_TRACE", "0") == "1"
    if trace:
        _install_ntff_hook()
    res = run_bass_kernel_spmd(nc, in_maps, core_ids=list(range(NCORES)), trace=trace)
    _STATE["last_results"] = res

    outs = []
    for r in res.results:
        oc = np.asarray(r["out"]).reshape(256, TPC)  # [c, t] bf16
        outs.append(oc.T.astype(np.float32))         # [t, c] f32
    out = np.concatenate(outs, axis=0)
    return out.reshape(b, s, 256)
